# revision 1
# baseline (speedup 1.0000x reference)
"""Cross-attention layer (QKV proj + softmax attention + out proj + residual
LayerNorm) on 8 Trainium2 NeuronCores, data-parallel over the batch.

Per-core program (one batch element):
  Q^T = WqT.T-matmuls over hs^T   (feature-on-partition layout)
  K^T similarly from cond^T
  V   natural layout, augmented per head with a mask-weight column so the
      softmax denominator falls out of the ctx matmul (row 64 of PSUM).
  scores^T[sk,sq] per head via row-packed pairs (two heads per 128-partition
      feature tile, tile_position rows 0-63 / 64-127, concurrent on PE).
  E = exp(scores/8) on ScalarE straight from PSUM, fp16.
  ctx^T = V_aug.T @ E accumulated over sk; normalize by 1/den; out-proj back
      to natural layout; residual + LayerNorm with bn_stats.
All matmul operands fp16 (1 cycle/row on PE), fp32 PSUM accumulation.
"""

import numpy as np
from contextlib import ExitStack

P = 128
H = 768
NH = 12
HD = 64
HT = H // P           # 6 feature tiles; one tile = 2 heads
NPH = HT              # head pairs


def _bcast(ap, n, bass):
    # Partition-broadcast view of a single-partition AP (step-0 partition dim).
    return bass.AP(
        tensor=ap.tensor,
        offset=ap.offset,
        ap=[[0, n]] + [list(d) for d in ap.ap[1:]],
    )


def build_nc(SQ=1024, SK=1024, repeat=1, dbg=False):
    import concourse.bass as bass
    import concourse.bacc as bacc
    import concourse.tile as tile
    from concourse import mybir

    f32 = mybir.dt.float32
    f16 = mybir.dt.float16
    Alu = mybir.AluOpType
    Act = mybir.ActivationFunctionType

    SQT = SQ // P
    SKT = SK // P

    nc = bacc.Bacc(trn_type="TRN2", debug=False)

    hsT = nc.dram_tensor("hsT", (H, SQ), f16, kind="ExternalInput")
    hsf = nc.dram_tensor("hsf", (SQ, H), f32, kind="ExternalInput")
    cdT = nc.dram_tensor("cdT", (H, SK), f16, kind="ExternalInput")
    wqT = nc.dram_tensor("wqT", (H, H), f16, kind="ExternalInput")
    wkT = nc.dram_tensor("wkT", (H, H), f16, kind="ExternalInput")
    wvT = nc.dram_tensor("wvT", (H, H), f16, kind="ExternalInput")
    woT = nc.dram_tensor("woT", (H, H), f16, kind="ExternalInput")
    bqv = nc.dram_tensor("bqv", (P, HT), f32, kind="ExternalInput")
    bkv = nc.dram_tensor("bkv", (P, HT), f32, kind="ExternalInput")
    bvr = nc.dram_tensor("bvr", (1, H), f16, kind="ExternalInput")
    bor = nc.dram_tensor("bor", (1, H), f16, kind="ExternalInput")
    wm = nc.dram_tensor("wm", (P, SKT), f32, kind="ExternalInput")
    wm12 = nc.dram_tensor("wm12", (P, SKT, NH), f16, kind="ExternalInput")
    sel = nc.dram_tensor("sel", (2, P), f16, kind="ExternalInput")
    lng = nc.dram_tensor("lng", (1, H), f32, kind="ExternalInput")
    lnb = nc.dram_tensor("lnb", (1, H), f32, kind="ExternalInput")
    out = nc.dram_tensor("out", (SQ, H), f32, kind="ExternalOutput")
    if dbg:
        dqT = nc.dram_tensor("dqT", (P, HT, SQ), f16, kind="ExternalOutput")
        dkT = nc.dram_tensor("dkT", (P, HT, SK), f16, kind="ExternalOutput")
        dva = nc.dram_tensor("dva", (P, SKT, NH, HD + 1), f16, kind="ExternalOutput")
        dE = nc.dram_tensor("dE", (2, P, SQ), f16, kind="ExternalOutput")
        dct = nc.dram_tensor("dct", (2, HD + 1, SQ), f32, kind="ExternalOutput")
        dbc = nc.dram_tensor("dbc", (P, SQ), f32, kind="ExternalOutput")

    PSW = max(SQ, SK, H)  # psum tile width

    def chunks(n, w=512):
        return [(i, min(i + w, n)) for i in range(0, n, w)]

    QC = chunks(SQ)
    KC = chunks(SK)
    HC = chunks(H)

    with tile.TileContext(nc) as tc, ExitStack() as ctx:
        const = ctx.enter_context(tc.tile_pool(name="const", bufs=1))
        pers = ctx.enter_context(tc.tile_pool(name="pers", bufs=1))
        work = ctx.enter_context(tc.tile_pool(name="work", bufs=2))
        ps = ctx.enter_context(tc.tile_pool(name="ps", bufs=1, space="PSUM"))

        # ---- constants / weights (loaded once, outside the repeat loop) ----
        wq_sb = const.tile([P, HT, H], f16, tag="wq")
        wk_sb = const.tile([P, HT, H], f16, tag="wk")
        wv_sb = const.tile([P, HT, H], f16, tag="wv")
        wo_sb = const.tile([P, HT, H], f16, tag="wo")
        for t in range(HT):
            nc.sync.dma_start(out=wq_sb[:, t, :], in_=wqT[t * P:(t + 1) * P, :])
            nc.sync.dma_start(out=wk_sb[:, t, :], in_=wkT[t * P:(t + 1) * P, :])
            nc.sync.dma_start(out=wv_sb[:, t, :], in_=wvT[t * P:(t + 1) * P, :])
            nc.sync.dma_start(out=wo_sb[:, t, :], in_=woT[t * P:(t + 1) * P, :])
        bq_sb = const.tile([P, HT], f32, tag="bq")
        bk_sb = const.tile([P, HT], f32, tag="bk")
        nc.sync.dma_start(out=bq_sb, in_=bqv[:, :])
        nc.sync.dma_start(out=bk_sb, in_=bkv[:, :])
        bv_sb = const.tile([1, H], f16, tag="bv")
        bo_sb = const.tile([1, H], f16, tag="bo")
        nc.sync.dma_start(out=bv_sb, in_=bvr[:, :])
        nc.sync.dma_start(out=bo_sb, in_=bor[:, :])
        wm_sb = const.tile([P, SKT], f32, tag="wm")
        nc.sync.dma_start(out=wm_sb, in_=wm[:, :])
        wm12_sb = const.tile([P, SKT, NH], f16, tag="wm12")
        nc.sync.dma_start(out=wm12_sb, in_=wm12[:, :, :])
        sel2 = const.tile([2, P], f16, tag="sel2")
        nc.sync.dma_start(out=sel2, in_=sel[:, :])
        ones1 = const.tile([1, P], f16, tag="ones1")
        nc.vector.memset(ones1, 1.0)
        eps_t = const.tile([P, 1], f32, tag="eps")
        nc.vector.memset(eps_t, 1e-5)
        g_sb = const.tile([P, H], f32, tag="g")
        b_sb = const.tile([P, H], f32, tag="b")
        nc.sync.dma_start(out=g_sb, in_=_bcast(lng.ap(), P, bass))
        nc.sync.dma_start(out=b_sb, in_=_bcast(lnb.ap(), P, bass))

        def body(_iv=None):
            # ---- load activations ----
            hsT_sb = pers.tile([P, HT, SQ], f16, tag="hsT")
            cdT_sb = pers.tile([P, HT, SK], f16, tag="cdT")
            for t in range(HT):
                nc.sync.dma_start(out=hsT_sb[:, t, :], in_=hsT[t * P:(t + 1) * P, :])
                nc.sync.dma_start(out=cdT_sb[:, t, :], in_=cdT[t * P:(t + 1) * P, :])

            qT_sb = pers.tile([P, HT, SQ], f16, tag="qT")
            kT_sb = pers.tile([P, HT, SK], f16, tag="kT")
            vaug = pers.tile([P, SKT, NH, HD + 1], f16, tag="vaug")

            # ---- Q^T / K^T projections ----
            for m in range(HT):
                qps = ps.tile([P, PSW], f32, tag=f"S{m % 2}")
                for k in range(HT):
                    for c0, c1 in QC:
                        nc.tensor.matmul(
                            qps[:, c0:c1],
                            lhsT=wq_sb[:, k, m * P:(m + 1) * P],
                            rhs=hsT_sb[:, k, c0:c1],
                            start=(k == 0), stop=(k == HT - 1),
                        )
                nc.vector.tensor_scalar(
                    out=qT_sb[:, m, :], in0=qps[:, 0:SQ], scalar1=bq_sb[:, m:m + 1],
                    scalar2=None, op0=Alu.add,
                )
                kps = ps.tile([P, PSW], f32, tag=f"C{m % 2}")
                for k in range(HT):
                    for c0, c1 in KC:
                        nc.tensor.matmul(
                            kps[:, c0:c1],
                            lhsT=wk_sb[:, k, m * P:(m + 1) * P],
                            rhs=cdT_sb[:, k, c0:c1],
                            start=(k == 0), stop=(k == HT - 1),
                        )
                nc.vector.tensor_scalar(
                    out=kT_sb[:, m, :], in0=kps[:, 0:SK], scalar1=bk_sb[:, m:m + 1],
                    scalar2=None, op0=Alu.add,
                )

            # ---- V (natural layout), bias via rank-1 matmul, mask-weighted ----
            for m in range(SKT):
                vps = ps.tile([P, PSW], f32, tag=f"S{m % 2}")
                for c0, c1 in HC:
                    for k in range(HT):
                        nc.tensor.matmul(
                            vps[:, c0:c1],
                            lhsT=cdT_sb[:, k, m * P:(m + 1) * P],
                            rhs=wv_sb[:, k, c0:c1],
                            start=(k == 0), stop=False,
                        )
                    nc.tensor.matmul(
                        vps[:, c0:c1], lhsT=ones1, rhs=bv_sb[:, c0:c1],
                        start=False, stop=True,
                    )
                nc.vector.tensor_scalar(
                    out=vaug[:, m, :, 0:HD],
                    in0=vps[:, 0:H].rearrange("p (n d) -> p n d", n=NH),
                    scalar1=wm_sb[:, m:m + 1], scalar2=None, op0=Alu.mult,
                )
                nc.vector.tensor_copy(out=vaug[:, m, :, HD], in_=wm12_sb[:, m, :])

            # ---- attention, one head pair (= one feature tile) at a time ----
            ctxT = pers.tile([P, HT, SQ], f16, tag="ctxT")
            for t in range(NPH):
                cps = [ps.tile([P, PSW], f32, tag=f"C{hi}", name=f"cps{hi}") for hi in range(2)]
                for m in range(SKT):
                    sps = [ps.tile([P, PSW], f32, tag=f"S{hi}", name=f"sps{hi}") for hi in range(2)]
                    e_sb = [
                        work.tile([P, SQ], f16, tag=f"E{hi}", bufs=3, name=f"E{hi}")
                        for hi in range(2)
                    ]
                    for hi in range(2):
                        lo, hh = 64 * hi, 64 * hi + 64
                        for c0, c1 in QC:
                            nc.tensor.matmul(
                                sps[hi][:, c0:c1],
                                lhsT=kT_sb[lo:hh, t, m * P:(m + 1) * P],
                                rhs=qT_sb[lo:hh, t, c0:c1],
                                start=True, stop=True,
                            )
                        nc.scalar.activation(
                            out=e_sb[hi], in_=sps[hi][:, 0:SQ],
                            func=Act.Exp, scale=1.0 / np.sqrt(HD),
                        )
                    if dbg and t == 0 and m == 0:
                        for hi in range(2):
                            nc.sync.dma_start(out=dE[hi, :, :], in_=e_sb[hi])
                    for hi in range(2):
                        h = 2 * t + hi
                        for c0, c1 in QC:
                            nc.tensor.matmul(
                                cps[hi][0:HD + 1, c0:c1],
                                lhsT=vaug[:, m, h, :],
                                rhs=e_sb[hi][:, c0:c1],
                                start=(m == 0), stop=(m == SKT - 1),
                            )
                # denominators -> reciprocal -> broadcast -> normalize
                ctmp = [
                    work.tile([HD + 1, SQ], f32, tag=f"ctmp{hi}", bufs=2, name=f"ctmp{hi}")
                    for hi in range(2)
                ]
                cshift = work.tile([P, SQ], f32, tag="cshift", bufs=2)
                dtmp = work.tile([2, SQ], f32, tag="dtmp", bufs=2, name="dtmp")
                rtmp = work.tile([2, SQ], f16, tag="rtmp", bufs=2, name="rtmp")
                for hi in range(2):
                    nc.vector.tensor_copy(out=ctmp[hi], in_=cps[hi][0:HD + 1, 0:SQ])
                    nc.sync.dma_start(
                        out=dtmp[hi:hi + 1, :], in_=ctmp[hi][HD:HD + 1, :]
                    )
                with nc.allow_low_precision(reason="1/den at fp16 for the broadcast matmul"):
                    nc.vector.reciprocal(out=rtmp, in_=dtmp)
                bcps = ps.tile([P, PSW], f32, tag="S0", name="bcps")
                for c0, c1 in QC:
                    nc.tensor.matmul(
                        bcps[:, c0:c1], lhsT=sel2, rhs=rtmp[:, c0:c1],
                        start=True, stop=True,
                    )
                nc.sync.dma_start(out=cshift[64:P, :], in_=ctmp[1][0:HD, :])
                if dbg and t == 0:
                    for hi in range(2):
                        nc.sync.dma_start(out=dct[hi, :, :], in_=ctmp[hi])
                    nc.sync.dma_start(out=dbc[:, :], in_=bc)
                nc.vector.tensor_tensor(
                    out=ctxT[0:64, t, :], in0=ctmp[0][0:HD, :],
                    in1=bcps[0:64, 0:SQ], op=Alu.mult,
                )
                nc.vector.tensor_tensor(
                    out=ctxT[64:P, t, :], in0=cshift[64:P, :],
                    in1=bcps[64:P, 0:SQ], op=Alu.mult,
                )

            if dbg:
                for tt in range(HT):
                    nc.sync.dma_start(out=dqT[:, tt, :], in_=qT_sb[:, tt, :])
                    nc.sync.dma_start(out=dkT[:, tt, :], in_=kT_sb[:, tt, :])
                for mm in range(SKT):
                    nc.sync.dma_start(out=dva[:, mm, :, :], in_=vaug[:, mm, :, :])
            # ---- out-projection + residual + LayerNorm (natural layout) ----
            for m in range(SQT):
                ops_ = ps.tile([P, PSW], f32, tag=f"S{m % 2}")
                for c0, c1 in HC:
                    for k in range(HT):
                        nc.tensor.matmul(
                            ops_[:, c0:c1],
                            lhsT=ctxT[:, k, m * P:(m + 1) * P],
                            rhs=wo_sb[:, k, c0:c1],
                            start=(k == 0), stop=False,
                        )
                    nc.tensor.matmul(
                        ops_[:, c0:c1], lhsT=ones1, rhs=bo_sb[:, c0:c1],
                        start=False, stop=True,
                    )
                hs_t = work.tile([P, H], f32, tag="hs", bufs=2)
                nc.sync.dma_start(out=hs_t, in_=hsf[m * P:(m + 1) * P, :])
                x_t = work.tile([P, H], f32, tag="x", bufs=2)
                nc.vector.tensor_tensor(
                    out=x_t, in0=ops_[:, 0:H], in1=hs_t, op=Alu.add
                )
                st = work.tile([P, 3, 6], f32, tag="st", bufs=2)
                for s in range(3):
                    nc.vector.bn_stats(
                        out=st[:, s, :], in_=x_t[:, s * 256:(s + 1) * 256]
                    )
                mv = work.tile([P, 2], f32, tag="mv", bufs=2)
                nc.vector.bn_aggr(out=mv, in_=st)
                nc.scalar.activation(
                    out=mv[:, 1:2], in_=mv[:, 1:2], func=Act.Sqrt,
                    bias=eps_t, scale=1.0,
                )
                nc.vector.reciprocal(out=mv[:, 1:2], in_=mv[:, 1:2])
                nc.vector.tensor_scalar(
                    out=x_t, in0=x_t, scalar1=mv[:, 0:1], scalar2=mv[:, 1:2],
                    op0=Alu.subtract, op1=Alu.mult,
                )
                nc.vector.tensor_tensor(out=x_t, in0=x_t, in1=g_sb, op=Alu.mult)
                nc.vector.tensor_tensor(out=x_t, in0=x_t, in1=b_sb, op=Alu.add)
                nc.sync.dma_start(out=out[m * P:(m + 1) * P, :], in_=x_t)

        if repeat == 1:
            body()
        else:
            with tc.For_i(0, repeat) as iv:
                body(iv)

    nc.compile()
    return nc


def prep_core_inputs(hs_b, cd_b, mask_b, Wq, bq, Wk, bk, Wv, bv, Wo, bo, ln_g, ln_b):
    """Host-side prep of one core's input map (numpy)."""
    f16 = np.float16
    f32 = np.float32
    SK = cd_b.shape[0]
    SKT = SK // P
    w = np.exp(-10000.0 * (1.0 - mask_b.astype(f32))).astype(f32)  # [SK]
    return {
        "hsT": np.ascontiguousarray(hs_b.T).astype(f16),
        "hsf": np.ascontiguousarray(hs_b).astype(f32),
        "cdT": np.ascontiguousarray(cd_b.T).astype(f16),
        "wqT": np.ascontiguousarray(Wq.T).astype(f16),
        "wkT": np.ascontiguousarray(Wk.T).astype(f16),
        "wvT": np.ascontiguousarray(Wv.T).astype(f16),
        "woT": np.ascontiguousarray(Wo.T).astype(f16),
        "bqv": np.ascontiguousarray(bq.reshape(HT, P).T).astype(f32),
        "bkv": np.ascontiguousarray(bk.reshape(HT, P).T).astype(f32),
        "bvr": bv.reshape(1, H).astype(f16),
        "bor": bo.reshape(1, H).astype(f16),
        "wm": np.ascontiguousarray(w.reshape(SKT, P).T).astype(f32),
        "wm12": np.ascontiguousarray(
            np.repeat(w.reshape(SKT, P).T[:, :, None], NH, axis=2)
        ).astype(f16),
        "sel": np.kron(np.eye(2), np.ones((1, 64))).astype(f16),
        "lng": ln_g.reshape(1, H).astype(f32),
        "lnb": ln_b.reshape(1, H).astype(f32),
    }


_NC_CACHE = {}


def kernel(hidden_states, condition_embeddings, condition_mask,
           Wq, bq, Wk, bk, Wv, bv, Wo, bo, ln_g, ln_b):
    from concourse.bass_utils import run_bass_kernel_spmd

    args = [np.asarray(a) for a in
            (hidden_states, condition_embeddings, condition_mask,
             Wq, bq, Wk, bk, Wv, bv, Wo, bo, ln_g, ln_b)]
    (hs, cd, mask, Wq, bq, Wk, bk, Wv, bv, Wo, bo, ln_g, ln_b) = args
    B, SQ, _ = hs.shape
    SK = cd.shape[1]

    key = (SQ, SK)
    if key not in _NC_CACHE:
        _NC_CACHE[key] = build_nc(SQ=SQ, SK=SK)
    nc = _NC_CACHE[key]

    in_maps = [
        prep_core_inputs(hs[b], cd[b], mask[b], Wq, bq, Wk, bk, Wv, bv,
                         Wo, bo, ln_g, ln_b)
        for b in range(B)
    ]
    res = run_bass_kernel_spmd(nc, in_maps, core_ids=list(range(B)))
    return np.stack([res.results[b]["out"] for b in range(B)], axis=0)



# revision 3
# speedup vs baseline: 1.4139x; 1.4139x over previous
"""Cross-attention layer (QKV proj + softmax attention + out proj + residual
LayerNorm) on 8 Trainium2 NeuronCores, data-parallel over the batch
(one batch element per core).

Design (measurement-driven, ~1.5x over the previous version on HW):
  - scores contraction zero-padded to K=128: lhsT = [kT_head; zeros], since
    K=64 matmuls stream at half rate on TRN2 and row-tiled pairing does not
    engage through this toolchain. Zero halves are memset once outside the
    repeat loop; per-iteration kT rows arrive via two small SBUF DMAs/tile.
  - M=65 augmented-V ctx matmul: the mask-weighted softmax denominator rides
    as a 65th output row of the same accumulation, costing zero extra PE
    cycles; one vanilla accumulation group per PSUM bank.
  - PSUM plan (8 banks): S0/S1 scores (per head, exp at [128,1024]) +
    CA/CB ctx accumulators; projection/out-proj phases reuse the same tags.
  - software-pipelined attention: scores of unit i+1 are emitted before ctx
    of unit i, so the PE never waits behind the exp chain; fast fp16 PSUM
    evacuation frees the accumulators before the 1/den round trip.
  - 1/den: fp16 reciprocal -> DRAM round-trip partition broadcast -> cheap
    fp16 tensor_tensor normalize; head1 ctx moved to partitions 64-127 with
    a single fp16 SBUF DMA.
  - LayerNorm on fp16 with bn_stats; output fp16, cast to f32 on the host.
  - host-side folds: V bias and out-proj bias into the residual
    (hsf := hs + bo + Wo@bv); all matmul operands fp16, fp32 PSUM.
"""

import numpy as np
from contextlib import ExitStack

P = 128
H = 768
NH = 12
HD = 64
HT = H // P           # 6 feature tiles; one tile = 2 heads
NPH = HT              # head pairs


def _bcast(ap, n, bass):
    # Partition-broadcast view of a single-partition AP (step-0 partition dim).
    return bass.AP(
        tensor=ap.tensor,
        offset=ap.offset,
        ap=[[0, n]] + [list(d) for d in ap.ap[1:]],
    )


def build_nc(SQ=1024, SK=1024, repeat=1, dbg=False):
    import concourse.bass as bass
    import concourse.bacc as bacc
    import concourse.tile as tile
    from concourse import mybir

    f32 = mybir.dt.float32
    f16 = mybir.dt.float16
    Alu = mybir.AluOpType
    Act = mybir.ActivationFunctionType

    SQT = SQ // P
    SKT = SK // P

    nc = bacc.Bacc(trn_type="TRN2", debug=False)

    hsT = nc.dram_tensor("hsT", (H, SQ), f16, kind="ExternalInput")
    hsf = nc.dram_tensor("hsf", (SQ, H), f16, kind="ExternalInput")
    cdT = nc.dram_tensor("cdT", (H, SK), f16, kind="ExternalInput")
    wqT = nc.dram_tensor("wqT", (H, H), f16, kind="ExternalInput")
    wkT = nc.dram_tensor("wkT", (H, H), f16, kind="ExternalInput")
    wvT = nc.dram_tensor("wvT", (H, H), f16, kind="ExternalInput")
    woT = nc.dram_tensor("woT", (H, H), f16, kind="ExternalInput")
    bqv = nc.dram_tensor("bqv", (P, HT), f32, kind="ExternalInput")
    bkv = nc.dram_tensor("bkv", (P, HT), f32, kind="ExternalInput")
    wmf = nc.dram_tensor("wmf", (P, SKT), f32, kind="ExternalInput")
    wm12 = nc.dram_tensor("wm12", (P, SKT, NH), f16, kind="ExternalInput")
    lng = nc.dram_tensor("lng", (1, H), f16, kind="ExternalInput")
    lnb = nc.dram_tensor("lnb", (1, H), f16, kind="ExternalInput")
    out = nc.dram_tensor("out", (SQ, H), f16, kind="ExternalOutput")
    scr = nc.dram_tensor("scr", (NPH, 2, SQ), f16, kind="Internal")

    def chunks(n, w=512):
        return [(i, min(i + w, n)) for i in range(0, n, w)]

    QC = chunks(SQ)
    HC = chunks(H)

    with tile.TileContext(nc) as tc, ExitStack() as ctx:
        const = ctx.enter_context(tc.tile_pool(name="const", bufs=1))
        pers = ctx.enter_context(tc.tile_pool(name="pers", bufs=1))
        work = ctx.enter_context(tc.tile_pool(name="work", bufs=2))
        ps = ctx.enter_context(tc.tile_pool(name="ps", bufs=1, space="PSUM"))

        # ---- constants / weights (loaded once, outside the repeat loop) ----
        wq_sb = const.tile([P, HT, H], f16, tag="wq")
        wk_sb = const.tile([P, HT, H], f16, tag="wk")
        wv_sb = const.tile([P, HT, H], f16, tag="wv")
        wo_sb = const.tile([P, HT, H], f16, tag="wo")
        for t in range(HT):
            nc.sync.dma_start(out=wq_sb[:, t, :], in_=wqT[t * P:(t + 1) * P, :])
            nc.sync.dma_start(out=wk_sb[:, t, :], in_=wkT[t * P:(t + 1) * P, :])
            nc.sync.dma_start(out=wv_sb[:, t, :], in_=wvT[t * P:(t + 1) * P, :])
            nc.sync.dma_start(out=wo_sb[:, t, :], in_=woT[t * P:(t + 1) * P, :])
        bq_sb = const.tile([P, HT], f32, tag="bq")
        bk_sb = const.tile([P, HT], f32, tag="bk")
        nc.sync.dma_start(out=bq_sb, in_=bqv[:, :])
        nc.sync.dma_start(out=bk_sb, in_=bkv[:, :])
        wmf_sb = const.tile([P, SKT], f32, tag="wmf")
        nc.sync.dma_start(out=wmf_sb, in_=wmf[:, :])
        wm12_sb = const.tile([P, SKT, NH], f16, tag="wm12")
        nc.sync.dma_start(out=wm12_sb, in_=wm12[:, :, :])
        eps_t = const.tile([P, 1], f32, tag="eps")
        nc.vector.memset(eps_t, 1e-5)
        g_sb = const.tile([P, H], f16, tag="g")
        b_sb = const.tile([P, H], f16, tag="b")
        nc.sync.dma_start(out=g_sb, in_=_bcast(lng.ap(), P, bass))
        nc.sync.dma_start(out=b_sb, in_=_bcast(lnb.ap(), P, bass))
        # zero-padded scores lhsT homes; the zero halves are written once
        # here and never touched again (per-iteration DMAs fill the live
        # halves only)
        kz0 = const.tile([P, HT, SK], f16, tag="kz0")
        kz1 = const.tile([P, HT, SK], f16, tag="kz1")
        nc.vector.memset(kz0, 0.0)
        nc.vector.memset(kz1, 0.0)

        def body(_iv=None):
            # ---- load activations (hsT on SP queue, cdT on ACT queue) ----
            hsT_sb = pers.tile([P, HT, SQ], f16, tag="hsT")
            cdT_sb = pers.tile([P, HT, SK], f16, tag="cdT")
            for t in range(HT):
                nc.sync.dma_start(out=hsT_sb[:, t, :], in_=hsT[t * P:(t + 1) * P, :])
                nc.scalar.dma_start(out=cdT_sb[:, t, :], in_=cdT[t * P:(t + 1) * P, :])

            qT_sb = pers.tile([P, HT, SQ], f16, tag="qT")
            kT_sb = pers.tile([P, HT, SK], f16, tag="kT")
            vaug = pers.tile([P, SKT, NH, HD + 1], f16, tag="vaug")

            # ---- Q^T / K^T projections (feature-on-partition layout) ----
            for m in range(HT):
                qps = ps.tile([P, SQ], f32, tag=f"A{m % 2}")
                for k in range(HT):
                    for c0, c1 in QC:
                        nc.tensor.matmul(
                            qps[:, c0:c1],
                            lhsT=wq_sb[:, k, m * P:(m + 1) * P],
                            rhs=hsT_sb[:, k, c0:c1],
                            start=(k == 0), stop=(k == HT - 1),
                        )
                nc.vector.tensor_scalar(
                    out=qT_sb[:, m, :], in0=qps[:, 0:SQ], scalar1=bq_sb[:, m:m + 1],
                    scalar2=None, op0=Alu.add,
                )
                kps = ps.tile([P, SK], f32, tag=f"B{m % 2}")
                for k in range(HT):
                    for c0, c1 in QC:
                        nc.tensor.matmul(
                            kps[:, c0:c1],
                            lhsT=wk_sb[:, k, m * P:(m + 1) * P],
                            rhs=cdT_sb[:, k, c0:c1],
                            start=(k == 0), stop=(k == HT - 1),
                        )
                nc.vector.tensor_scalar(
                    out=kT_sb[:, m, :], in0=kps[:, 0:SK], scalar1=bk_sb[:, m:m + 1],
                    scalar2=None, op0=Alu.add,
                )
                # assemble the zero-padded scores lhsT (live halves only)
                nc.sync.dma_start(out=kz0[0:64, m, :], in_=kT_sb[0:64, m, :])
                nc.sync.dma_start(out=kz1[64:P, m, :], in_=kT_sb[64:P, m, :])

            # ---- V (natural layout), mask-weighted, den column appended ----
            for m in range(SKT):
                vps = ps.tile([P, SQ], f32, tag=f"A{m % 2}")
                for k in range(HT):
                    for c0, c1 in HC:
                        nc.tensor.matmul(
                            vps[:, c0:c1],
                            lhsT=cdT_sb[:, k, m * P:(m + 1) * P],
                            rhs=wv_sb[:, k, c0:c1],
                            start=(k == 0), stop=(k == HT - 1),
                        )
                nc.vector.tensor_scalar(
                    out=vaug[:, m, :, 0:HD],
                    in0=vps[:, 0:H].rearrange("p (n d) -> p n d", n=NH),
                    scalar1=wmf_sb[:, m:m + 1], scalar2=None, op0=Alu.mult,
                )
                nc.vector.tensor_copy(out=vaug[:, m, :, HD], in_=wm12_sb[:, m, :])

            # ---- attention: units (t, m), scores software-pipelined by one ----
            ctxT = pers.tile([P, HT, SQ], f16, tag="ctxT")
            units = [(t, m) for t in range(NPH) for m in range(SKT)]

            def scores_mms(t, m):
                s0 = ps.tile([P, SQ], f32, tag="A0", name=f"s0_{t}_{m}")
                s1 = ps.tile([P, SQ], f32, tag="A1", name=f"s1_{t}_{m}")
                for c0, c1 in QC:
                    nc.tensor.matmul(
                        s0[:, c0:c1],
                        lhsT=kz0[:, t, m * P:(m + 1) * P],
                        rhs=qT_sb[:, t, c0:c1],
                        start=True, stop=True,
                    )
                for c0, c1 in QC:
                    nc.tensor.matmul(
                        s1[:, c0:c1],
                        lhsT=kz1[:, t, m * P:(m + 1) * P],
                        rhs=qT_sb[:, t, c0:c1],
                        start=True, stop=True,
                    )
                return s0, s1

            spair = scores_mms(*units[0])
            CA = CB = None
            for i, (t, m) in enumerate(units):
                if m == 0:
                    CA = ps.tile([P, SQ], f32, tag="B0", name=f"ca{t}")
                    CB = ps.tile([P, SQ], f32, tag="B1", name=f"cb{t}")
                s0, s1 = spair
                e0 = work.tile([P, SQ], f16, tag="E0", bufs=4, name=f"e0_{t}_{m}")
                e1 = work.tile([P, SQ], f16, tag="E1", bufs=4, name=f"e1_{t}_{m}")
                nc.scalar.activation(
                    out=e0, in_=s0[:, 0:SQ], func=Act.Exp, scale=1.0 / np.sqrt(HD)
                )
                nc.scalar.activation(
                    out=e1, in_=s1[:, 0:SQ], func=Act.Exp, scale=1.0 / np.sqrt(HD)
                )
                # next unit's scores ahead of this unit's ctx (keeps PE fed)
                if i + 1 < len(units):
                    spair = scores_mms(*units[i + 1])
                h0, h1 = 2 * t, 2 * t + 1
                first, last = (m == 0), (m == SKT - 1)
                for c0, c1 in QC:
                    nc.tensor.matmul(
                        CA[0:HD + 1, c0:c1], lhsT=vaug[:, m, h0, :],
                        rhs=e0[:, c0:c1], start=first, stop=last,
                    )
                for c0, c1 in QC:
                    nc.tensor.matmul(
                        CB[0:HD + 1, c0:c1], lhsT=vaug[:, m, h1, :],
                        rhs=e1[:, c0:c1], start=first, stop=last,
                    )
                if last:
                    # fast fp16 evacuation frees CA/CB for the next head pair;
                    # 1/den -> DRAM round-trip broadcast -> cheap fp16 TTs
                    cua = work.tile([P, SQ], f16, tag="cua", bufs=2, name=f"cua{t}")
                    cub = work.tile([P, SQ], f16, tag="cub", bufs=2, name=f"cub{t}")
                    nc.vector.tensor_copy(out=cua[0:HD + 1, :], in_=CA[0:HD + 1, 0:SQ])
                    nc.vector.tensor_copy(out=cub[0:HD + 1, :], in_=CB[0:HD + 1, 0:SQ])
                    rta = work.tile([P, SQ], f16, tag="rta", bufs=2, name=f"rta{t}")
                    rtb = work.tile([P, SQ], f16, tag="rtb", bufs=2, name=f"rtb{t}")
                    with nc.allow_low_precision(reason="1/den at fp16"):
                        nc.vector.reciprocal(
                            out=rta[HD:HD + 1, :], in_=cua[HD:HD + 1, :]
                        )
                        nc.vector.reciprocal(
                            out=rtb[HD:HD + 1, :], in_=cub[HD:HD + 1, :]
                        )
                    nc.sync.dma_start(out=scr[t, 0:1, :], in_=rta[HD:HD + 1, :])
                    nc.sync.dma_start(out=scr[t, 1:2, :], in_=rtb[HD:HD + 1, :])
                    rba = work.tile([HD, SQ], f16, tag="rba", bufs=2, name=f"rba{t}")
                    rbb = work.tile([HD, SQ], f16, tag="rbb", bufs=2, name=f"rbb{t}")
                    nc.sync.dma_start(
                        out=rba[0:HD, :], in_=_bcast(scr[t, 0:1, :], HD, bass)
                    )
                    nc.sync.dma_start(
                        out=rbb[0:HD, :], in_=_bcast(scr[t, 1:2, :], HD, bass)
                    )
                    nc.vector.tensor_tensor(
                        out=ctxT[0:HD, t, :], in0=cua[0:HD, :], in1=rba,
                        op=Alu.mult,
                    )
                    ch1 = work.tile([HD, SQ], f16, tag="ch1", bufs=2, name=f"ch1{t}")
                    nc.vector.tensor_tensor(
                        out=ch1, in0=cub[0:HD, :], in1=rbb, op=Alu.mult,
                    )
                    nc.sync.dma_start(out=ctxT[HD:P, t, :], in_=ch1)

            # ---- out-projection + residual + LayerNorm (natural layout) ----
            for m in range(SQT):
                ops_ = ps.tile([P, SQ], f32, tag=f"A{m % 2}")
                for c0, c1 in HC:
                    for k in range(HT):
                        nc.tensor.matmul(
                            ops_[:, c0:c1],
                            lhsT=ctxT[:, k, m * P:(m + 1) * P],
                            rhs=wo_sb[:, k, c0:c1],
                            start=(k == 0), stop=(k == HT - 1),
                        )
                hs_t = work.tile([P, H], f16, tag="hs", bufs=3)
                nc.sync.dma_start(out=hs_t, in_=hsf[m * P:(m + 1) * P, :])
                x_t = work.tile([P, H], f16, tag="x", bufs=3)
                nc.vector.tensor_tensor(
                    out=x_t, in0=ops_[:, 0:H], in1=hs_t, op=Alu.add
                )
                st = work.tile([P, 3, 6], f32, tag="st", bufs=3)
                for s in range(3):
                    nc.vector.bn_stats(
                        out=st[:, s, :], in_=x_t[:, s * 256:(s + 1) * 256]
                    )
                mv = work.tile([P, 2], f32, tag="mv", bufs=3)
                nc.vector.bn_aggr(out=mv, in_=st)
                nc.scalar.activation(
                    out=mv[:, 1:2], in_=mv[:, 1:2], func=Act.Sqrt,
                    bias=eps_t, scale=1.0,
                )
                nc.vector.reciprocal(out=mv[:, 1:2], in_=mv[:, 1:2])
                xn = work.tile([P, H], f16, tag="xn", bufs=3)
                nc.vector.tensor_scalar(
                    out=xn, in0=x_t, scalar1=mv[:, 0:1], scalar2=mv[:, 1:2],
                    op0=Alu.subtract, op1=Alu.mult,
                )
                nc.vector.tensor_tensor(out=xn, in0=xn, in1=g_sb, op=Alu.mult)
                xf = work.tile([P, H], f16, tag="xf", bufs=3)
                nc.vector.tensor_tensor(out=xf, in0=xn, in1=b_sb, op=Alu.add)
                nc.sync.dma_start(out=out[m * P:(m + 1) * P, :], in_=xf)

        if repeat == 1:
            body()
        else:
            with tc.For_i(0, repeat,
                          hint_engines=(mybir.EngineType.PE,)) as iv:
                body(iv)

    nc.compile()
    return nc


def prep_core_inputs(hs_b, cd_b, mask_b, Wq, bq, Wk, bk, Wv, bv, Wo, bo, ln_g, ln_b):
    """Host-side prep of one core's input map (numpy)."""
    f16 = np.float16
    f32 = np.float32
    SK = cd_b.shape[0]
    SKT = SK // P
    w = np.exp(-10000.0 * (1.0 - mask_b.astype(f32))).astype(f32)  # [SK]
    # fold out-proj bias and V bias into the residual:
    # hs + ctx@Wo.T + bo, ctx = ctx0 + bv  ->  (hs + bo + Wo@bv) + ctx0@Wo.T
    bo2 = bo.astype(f32) + Wo.astype(f32) @ bv.astype(f32)
    wcol = np.ascontiguousarray(w.reshape(SKT, P).T)
    return {
        "hsT": np.ascontiguousarray(hs_b.T).astype(f16),
        "hsf": (np.ascontiguousarray(hs_b) + bo2[None, :]).astype(f16),
        "cdT": np.ascontiguousarray(cd_b.T).astype(f16),
        "wqT": np.ascontiguousarray(Wq.T).astype(f16),
        "wkT": np.ascontiguousarray(Wk.T).astype(f16),
        "wvT": np.ascontiguousarray(Wv.T).astype(f16),
        "woT": np.ascontiguousarray(Wo.T).astype(f16),
        "bqv": np.ascontiguousarray(bq.reshape(HT, P).T).astype(f32),
        "bkv": np.ascontiguousarray(bk.reshape(HT, P).T).astype(f32),
        "wmf": wcol.astype(f32),
        "wm12": np.ascontiguousarray(
            np.repeat(wcol[:, :, None], NH, axis=2)
        ).astype(f16),
        "lng": ln_g.reshape(1, H).astype(f16),
        "lnb": ln_b.reshape(1, H).astype(f16),
    }


_NC_CACHE = {}


def kernel(hidden_states, condition_embeddings, condition_mask,
           Wq, bq, Wk, bk, Wv, bv, Wo, bo, ln_g, ln_b):
    from concourse.bass_utils import run_bass_kernel_spmd

    args = [np.asarray(a) for a in
            (hidden_states, condition_embeddings, condition_mask,
             Wq, bq, Wk, bk, Wv, bv, Wo, bo, ln_g, ln_b)]
    (hs, cd, mask, Wq, bq, Wk, bk, Wv, bv, Wo, bo, ln_g, ln_b) = args
    B, SQ, _ = hs.shape
    SK = cd.shape[1]

    key = (SQ, SK)
    if key not in _NC_CACHE:
        _NC_CACHE[key] = build_nc(SQ=SQ, SK=SK)
    nc = _NC_CACHE[key]

    in_maps = [
        prep_core_inputs(hs[b], cd[b], mask[b], Wq, bq, Wk, bk, Wv, bv,
                         Wo, bo, ln_g, ln_b)
        for b in range(B)
    ]
    res = run_bass_kernel_spmd(nc, in_maps, core_ids=list(range(B)))
    return np.stack(
        [res.results[b]["out"].astype(np.float32) for b in range(B)], axis=0
    )
